# revision 36
# baseline (speedup 1.0000x reference)
"""Trainium2 Bass kernel for the MLPConstructor2 adjacency problem.

Computes, per batch b (one NeuronCore each, 8-way data parallel over B):
    adj[i, j] = tanh(relu(x1_i @ w1 + x2_j @ w2 + b))
for the four (spatial/temporal) quadrants of a (2560, 2560) output.

v5 design (ACT-bound, ~46us ScalarE floor):
- Output is stored as fp16 (tolerance is 2e-2; fp16 adds ~1e-3), halving
  the HBM store traffic to 13.1 MB/core. tanh(relu(x)) == relu(tanh(x)),
  so relu runs first (fused on VectorE) and tanh's fp16 result stores
  directly.
- x is staged ONCE per node set, in (p t) layout; both the col stats and
  the row scalars come from it. Output blocks are row-strided to match:
  block t covers rows {16p+t} u {16p+t+8}, so the per-partition row
  scalar is stat slot t (resp. t+8) with no second staging layout.
- Per block: 4 fused VectorE tensor_scalar (add row scalar, max 0) fill
  an fp16 tmp [128, 5120]; ScalarE runs ONE tanh over it (fp16 in/out);
  one 1.31 MB store writes the 256 strided rows (256 x 5120 B descs).
- All weights+biases arrive pre-packed in one "wpack" input (host-side
  concat), loaded with a single partition-broadcast DMA. Quadrant biases
  fold into the row stats (off the col critical path).
- Col stats round-trip through a DRAM scratch and return partition-
  broadcast; the spatial chain rides the Sync ring, the temporal chain
  the Scalar ring, so the two overlap. A dummy tanh at t=0 pulls the
  ACT_TABLE_LOAD off the critical path.
"""

import numpy as np
from contextlib import ExitStack

import concourse.bacc as bacc
import concourse.mybir as mybir
import concourse.tile as tile
from concourse.bass_utils import run_bass_kernel_spmd

B, N, T, D = 8, 2048, 512, 32
W = N + T                     # 2560
NT, TT = N // 128, T // 128   # 16, 4 stat slots per partition
F32 = mybir.dt.float32
F16 = mybir.dt.float16
QUADS = ("ss", "st", "ts", "tt")


def _emit(tc, sp, tm, wp_in, scr, adj):
    nc = tc.nc
    AF = mybir.ActivationFunctionType
    OP = mybir.AluOpType
    with ExitStack() as ctx:
        ctx.enter_context(nc.allow_low_precision(
            reason="fp16 intermediates; tolerance is 2e-2, fp16 adds ~1e-3"
        ))
        const = ctx.enter_context(tc.tile_pool(name="const", bufs=1))
        tmpp = ctx.enter_context(tc.tile_pool(name="tmpp", bufs=2))
        outp = ctx.enter_context(tc.tile_pool(name="outp", bufs=3))

        # ---- one broadcast load for all weights + biases ------------------
        # wpack = [w_ss | w_st | w_ts | w_tt | b_ss b_st b_ts b_tt] (260 f32)
        wp = const.tile([128, 260], F32)
        nc.scalar.dma_start(wp[:], wp_in.unsqueeze(0).broadcast_to((128, 260)))

        def w_row(q):  # first half of w_q: row-side weights
            return wp[:, 64 * q : 64 * q + D]

        def w_col(q):  # second half: col-side weights
            return wp[:, 64 * q + D : 64 * q + 2 * D]

        def b_q(q):
            return wp[:, 256 + q : 257 + q]

        # ---- ACT table-load warmup (overlaps the wpack transfer) ----------
        warm = const.tile([128, 1], F32)
        nc.vector.memset(warm[:], 0.0)
        nc.scalar.activation(warm[:], warm[:], AF.Tanh)

        # ---- stage inputs, (p t) layout: row p*nt+t at [p, t*D:(t+1)*D] ---
        # x_tm first: it is 4x smaller and gates the quick T-part stat chain.
        x_tm = const.tile([128, TT * D], F32)
        nc.sync.dma_start(x_tm[:], tm.rearrange("(p t) d -> p t d", p=128))
        x_sp = const.tile([128, NT * D], F32)
        nc.sync.dma_start(x_sp[:], sp.rearrange("(p t) d -> p t d", p=128))


        # ---- bisect probe: transposes + shifted loads (results unused) ----
        x_tm_sh = const.tile([64, TT * D], F32)
        nc.scalar.dma_start(
            x_tm_sh[:], tm.rearrange("(p t) d -> p t d", p=128)[64:128]
        )
        x_sp_sh = const.tile([64, NT * D], F32)
        nc.scalar.dma_start(
            x_sp_sh[:], sp.rearrange("(p t) d -> p t d", p=128)[64:128]
        )
        xT_sp = const.tile([64, NT * D], F32, name="xT_sp", tag="xT_sp")
        nc.vector.transpose(xT_sp[:], x_sp[0:64, :])
        xT_sph = const.tile([64, NT * D], F32, name="xT_sph", tag="xT_sph")
        nc.vector.transpose(xT_sph[:], x_sp_sh[:])
        xT_tm = const.tile([64, TT * D], F32, name="xT_tm", tag="xT_tm")
        nc.vector.transpose(xT_tm[:], x_tm[0:64, :])
        xT_tmh = const.tile([64, TT * D], F32, name="xT_tmh", tag="xT_tmh")
        nc.vector.transpose(xT_tmh[:], x_tm_sh[:])


        # ---- bisect probe B: PE matmuls + psum copies (results unused) ----
        psum = ctx.enter_context(tc.tile_pool(name="psum", bufs=1, space="PSUM"))
        wT = const.tile([128, 4 * D], F32)
        for q in range(4):
            nc.vector.transpose(
                wT[:, 32 * q : 32 * (q + 1)], wp[:, 64 * q + D : 64 * q + 2 * D]
            )
        ones = const.tile([128, 128], F32)
        nc.vector.memset(ones[:], 1.0)
        wmat = const.tile([128, 4 * 128], F32)   # q at [128q : 128(q+1)]
        for q in range(4):
            nc.vector.tensor_scalar(
                wmat[:, 128 * q : 128 * (q + 1)], ones[:],
                wT[:, 32 * q : 32 * q + 1], None, OP.mult,
            )
        colx = const.tile([128, W], F16, name="colx", tag="colx")
        psn = psum.tile([128, N], F32, name="psn_x", tag="psn")
        for a in range(4):
            xs = (xT_sp, xT_sph)[a // 2]
            b = 32 * (a % 2)
            nc.tensor.matmul(
                psn[:, 512 * a : 512 * (a + 1)],
                wmat[b : b + 32, 0:128],
                xs[b : b + 32, :],
            )
        nc.vector.tensor_copy(colx[:, 0:N], psn[:])

        # ---- stats on VectorE: mul + reduce over D ------------------------
        def mulred(x, nt, w, dst, name):
            prod = const.tile([128, nt * D], F32, name=f"prod_{name}", tag="prod")
            x3 = x[:].rearrange("p (t d) -> p t d", t=nt)
            p3 = prod[:].rearrange("p (t d) -> p t d", t=nt)
            w3 = w.unsqueeze(1).broadcast_to((128, nt, D))
            nc.vector.tensor_tensor(p3, x3, w3, OP.mult)
            nc.vector.tensor_reduce(dst, p3, axis=mybir.AxisListType.X, op=OP.add)

        # col stats for one output half (ss+st -> col_sp, ts+tt -> col_tm):
        # fp16 [128, W] rebuilt via DRAM scratch + partition-broadcast.
        # dma_eng picks the HWDGE ring so the two halves' chains overlap.
        # col stats for one output half (ss+st -> col_sp, ts+tt -> col_tm):
        # fp16 [128, W] rebuilt via DRAM scratch + partition-broadcast.
        # The small T-part (from x_tm) runs its whole chain first so its
        # two DMA-hop latencies overlap the larger N-part's compute.
        # dma_eng picks the HWDGE ring so the two halves' chains overlap.
        def col_half(q_sp, q_tm, scr_t, dma_eng, name):
            cst = const.tile([128, NT + TT], F16, name=f"cst_{name}", tag=f"cst_{name}")
            col = const.tile([128, W], F16, name=f"col_{name}", tag=f"col_{name}")
            mulred(x_tm, TT, w_col(q_tm), cst[:, NT:], f"c{name}t")
            dma_eng.dma_start(
                scr_t[N:W].rearrange("(p j) -> p j", p=128), cst[:, NT:]
            )
            dma_eng.dma_start(
                col[:, N:W], scr_t[N:W].unsqueeze(0).broadcast_to((128, T))
            )
            mulred(x_sp, NT, w_col(q_sp), cst[:, 0:NT], f"c{name}s")
            dma_eng.dma_start(
                scr_t[0:N].rearrange("(p j) -> p j", p=128), cst[:, 0:NT]
            )
            dma_eng.dma_start(
                col[:, 0:N], scr_t[0:N].unsqueeze(0).broadcast_to((128, N))
            )
            return col

        col_sp = col_half(0, 1, scr["sp"], nc.sync, "sp")    # w_ss2, w_st2

        # row stats (slot t = row p*nt + t), quadrant biases folded in:
        # r_sp = [a_ss + b_ss | a_st + b_st], r_tm = [a_ts + b_ts | a_tt + b_tt]
        r_sp = const.tile([128, 2 * NT], F32)
        mulred(x_sp, NT, w_row(0), r_sp[:, 0:NT], "r_ss")
        nc.vector.tensor_scalar_add(r_sp[:, 0:NT], r_sp[:, 0:NT], b_q(0))
        mulred(x_sp, NT, w_row(1), r_sp[:, NT:], "r_st")
        nc.vector.tensor_scalar_add(r_sp[:, NT:], r_sp[:, NT:], b_q(1))

        # gpsimd (SWDGE) is slower but idle, and col_tm has ~40us of slack;
        # keeping it off Sync/Scalar keeps block 0's TANH path clear.
        col_tm = col_half(2, 3, scr["tm"], nc.gpsimd, "tm")  # w_ts2, w_tt2

        r_tm = const.tile([128, 2 * TT], F32)
        mulred(x_tm, TT, w_row(2), r_tm[:, 0:TT], "r_ts")
        nc.vector.tensor_scalar_add(r_tm[:, 0:TT], r_tm[:, 0:TT], b_q(2))
        mulred(x_tm, TT, w_row(3), r_tm[:, TT:], "r_tt")
        nc.vector.tensor_scalar_add(r_tm[:, TT:], r_tm[:, TT:], b_q(3))

        # ---- main loop: strided 256-row blocks -----------------------------
        # spatial block t (t=0..7): rows {16p+t} (h=0) and {16p+t+8} (h=1)
        # temporal block t (t=0..1): rows 2048 + {4p+t} and 2048 + {4p+t+2}
        def block(k, t, base, nt, col, rst, hs, split=False):
            tmp = tmpp.tile([128, 2 * W], F16, name=f"tmp{k}", tag="tmp")
            ot = outp.tile([128, 2 * W], F16, name=f"ot{k}", tag="ot")
            quad = adj[base : base + 128 * nt, :]
            for h in range(2):
                o = h * W
                s = t + h * hs
                rn = rst[:, s : s + 1]
                rt = rst[:, nt + s : nt + s + 1]
                nc.vector.tensor_scalar(
                    tmp[:, o + N : o + W], col[:, N:W], rt, 0.0, OP.add, OP.max
                )
                nc.vector.tensor_scalar(
                    tmp[:, o : o + N], col[:, 0:N], rn, 0.0, OP.add, OP.max
                )
                if split:
                    nc.scalar.activation(ot[:, o : o + W], tmp[:, o : o + W], AF.Tanh)
                    nc.sync.dma_start(
                        quad.rearrange("(p r) w -> p r w", p=128)[:, s : s + 1, :],
                        ot[:, o : o + W].rearrange("p (r w) -> p r w", r=1),
                    )
            if not split:
                nc.scalar.activation(ot[:], tmp[:], AF.Tanh)
                # partition p -> rows base + nt*p + t and base + nt*p + t + hs
                nc.sync.dma_start(
                    quad.rearrange("(p g r) w -> p g r w", p=128, g=2)[
                        :, :, t : t + 1, :
                    ],
                    ot[:].rearrange("p (g w) -> p g w", g=2).unsqueeze(2),
                )

        for t in range(NT // 2):
            block(t, t, 0, NT, col_sp, r_sp, NT // 2, split=(t == 0))
        for t in range(TT // 2):
            block(8 + t, t, N, TT, col_tm, r_tm, TT // 2,
                  split=(t == TT // 2 - 1))


def build_nc(num_devices=8):
    nc = bacc.Bacc(
        "TRN2",
        target_bir_lowering=False,
        debug=False,
        enable_asserts=True,
        num_devices=num_devices,
    )
    sp = nc.dram_tensor("spatial_nodes", (N, D), F32, kind="ExternalInput").ap()
    tm = nc.dram_tensor("temporal_nodes", (T, D), F32, kind="ExternalInput").ap()
    wp = nc.dram_tensor("wpack", (260,), F32, kind="ExternalInput").ap()
    scr = {
        "sp": nc.dram_tensor("scr_sp", (W,), F16, kind="Internal").ap(),
        "tm": nc.dram_tensor("scr_tm", (W,), F16, kind="Internal").ap(),
    }
    adj = nc.dram_tensor("adj", (W, W), F16, kind="ExternalOutput").ap()

    with tile.TileContext(nc) as tc:
        _emit(tc, sp, tm, wp, scr, adj)
    nc.compile()
    return nc


def make_in_maps(inputs):
    wpack = np.concatenate(
        [np.asarray(inputs[f"w_{nm}"], np.float32).reshape(-1) for nm in QUADS]
        + [np.asarray(inputs[f"b_{nm}"], np.float32).reshape(-1) for nm in QUADS]
    )
    in_maps = []
    for b in range(B):
        m = {
            "spatial_nodes": np.ascontiguousarray(inputs["spatial_nodes"][b], np.float32),
            "temporal_nodes": np.ascontiguousarray(inputs["temporal_nodes"][b], np.float32),
            "wpack": wpack,
        }
        in_maps.append(m)
    return in_maps


_NC = {}


def run(inputs, trace=False, trace_cores=None):
    if 8 not in _NC:
        _NC[8] = build_nc(8)
    res = run_bass_kernel_spmd(
        _NC[8], make_in_maps(inputs), core_ids=list(range(B)), trace=trace,
        trace_cores=trace_cores,
    )
    out = np.stack(
        [res.results[i]["adj"].astype(np.float32) for i in range(B)], axis=0
    )
    return out, res


def kernel(**inputs) -> np.ndarray:
    out, _ = run(inputs, trace=False)
    return out
